# revision 3
# baseline (speedup 1.0000x reference)
"""Causal self-attention (B=2, T=2048, C=1024, H=16) on 8 NeuronCores.

Sharding: data-parallel over batch (2) x tensor-parallel over heads
(4 groups of 4 heads). Each core computes q/k/v projections for its
head slice, causal attention for its 4 heads, and a partial c_proj
([2048,256] @ [256,1024]); the host sums the 4 partials per batch
(the "all-reduce") and folds the v/proj biases in at the end.

The reference uses scale = float32(C // (H ** -0.5)) = 4096.0, so
logits are huge and softmax is near-one-hot; q/k and Q@K^T need ~22
bits of score precision (min top-2 scaled-logit gap ~0.2). Both are
computed with fp16 hi/lo splits and exact fp16xfp16->fp32 products:

  q/k projection: 3 passes (xh@wh, xh@wl, xl@wh), fp32 PSUM accum.
  scores:         2 matmuls per key chunk instead of 3 --
                  hh:    K=64  qh . kh
                  cross: K=128 [qh;ql] . [kl;kh] = qh.kl + ql.kh
                  (the dropped ql.kl term is ~2^-22 relative, same as
                  the baseline 3-pass scheme, but 33% fewer PE
                  columns.)

Scores run per head on 128-query blocks (klen = 128*(Qb+1) exactly,
no padded columns); softmax stats are per chunk of 512 keys (DVE max,
ACT exp with accumulate); P is normalized in-place on the otherwise
idle GPSIMD engine and transposed by the DMA xbar (single queue --
concurrent xbar transposes corrupt each other). PV uses V as the
stationary operand; the two heads of a pair write the [0:64) and
[64:128) partition ranges of one PSUM tile (tile_position), so aout
keeps the [128 vdims, tokens] pair layout c_proj consumes with K=128.

Score operand tiles per head (fp16, assembled from projection PSUM by
ACT/DVE extracts plus SBUF->SBUF partition-move DMAs):
  q2[h]  [128,T]: rows 0:64 qh, rows 64:128 ql
  kx[h]  [128,T]: rows 0:64 kl, rows 64:128 kh
  khh[h] [64,T]:  kh again (hh needs kh on the same partitions as qh)

Per-core DRAM tensors:
  xTh  [1024, 2048] f16  x[b] transposed, high fp16 half
  xTl  [1024, 2048] f16  low half (x - xTh)
  wqkh/wqkl [1024, 512] f16  cols [q_h0..q_h3 | k_h0..k_h3]
  bqk  [512, 1]     f32  matching bias layout
  wv   [1024, 256]  f16  v columns for the head group
  wp   [256, 1024]  f16  w_proj rows for the head group
  outT [1024, 2048] f16  partial output, transposed (host sums in f32)
"""

import numpy as np

import concourse.bacc as bacc
import concourse.mybir as mybir
import concourse.tile as tile
from concourse.bass_utils import run_bass_kernel_spmd
from concourse.masks import make_causal_mask

f32 = mybir.dt.float32
f16 = mybir.dt.float16
AF = mybir.ActivationFunctionType
AX = mybir.AxisListType
ALU = mybir.AluOpType

B, T, C = 2, 2048, 1024
H, HS = 16, 64
NCORES = 8
HG = 4            # head groups (cores per batch)
NHL = H // HG     # local heads per core = 4
P = 128
KT = C // P       # 8 contraction tiles
CH = 512          # free-dim chunk
NT = T // CH      # 4 token chunks
NQB = T // P      # 16 query blocks of 128
WQ = 2 * NHL * HS  # 512 q+k columns per core
SCALE = 4096.0    # float32(C // (H ** -0.5)) -- faithful to source bug
MASK_VAL = -1e10

_CACHE = {}


def _build_program():
    nc = bacc.Bacc("TRN2", target_bir_lowering=False, debug=False,
                   num_devices=NCORES)
    xTh = nc.dram_tensor("xTh", [C, T], f16, kind="ExternalInput").ap()
    xTl = nc.dram_tensor("xTl", [C, T], f16, kind="ExternalInput").ap()
    wqkh = nc.dram_tensor("wqkh", [C, WQ], f16, kind="ExternalInput").ap()
    wqkl = nc.dram_tensor("wqkl", [C, WQ], f16, kind="ExternalInput").ap()
    bqk = nc.dram_tensor("bqk", [WQ, 1], f32, kind="ExternalInput").ap()
    wv = nc.dram_tensor("wv", [C, NHL * HS], f16, kind="ExternalInput").ap()
    wp = nc.dram_tensor("wp", [NHL * HS, C], f16, kind="ExternalInput").ap()
    outT = nc.dram_tensor("outT", [C, T], f16, kind="ExternalOutput").ap()

    with tile.TileContext(nc) as tc:
        with (
            tc.tile_pool(name="const", bufs=1) as const,
            tc.tile_pool(name="wts", bufs=1) as wts,
            tc.tile_pool(name="xin", bufs=2) as xin,
            tc.tile_pool(name="qk", bufs=1) as qkp,
            tc.tile_pool(name="stg", bufs=2) as stg,
            tc.tile_pool(name="vsb", bufs=1) as vsb,
            tc.tile_pool(name="att", bufs=6) as att,
            tc.tile_pool(name="ptp", bufs=8) as ptp,
            tc.tile_pool(name="aout", bufs=1) as aout,
            tc.tile_pool(name="stage", bufs=2) as stage,
            tc.tile_pool(name="ps_big", bufs=6, space="PSUM") as ps_big,
            tc.tile_pool(name="ps_o", bufs=2, space="PSUM") as ps_o,
        ):
            # [128, 128] causal mask for the diagonal key block
            mask = const.tile([P, P], f32, tag="mask")
            make_causal_mask(nc, mask[:], mask_val=MASK_VAL)

            # ---- batched loads: one DMA per tensor kt-half per chunk
            def xload_one(src, tag, nt):
                t_ = xin.tile([P, KT * CH], f16, tag=tag, name=f"{tag}_{nt}")
                hk = KT // 2
                for h in range(2):
                    nc.sync.dma_start(
                        t_[:, h * hk * CH:(h + 1) * hk * CH]
                        .rearrange("p (k c) -> p k c", c=CH),
                        src.rearrange("(k p) t -> p k t", p=P)
                           [:, h * hk:(h + 1) * hk, nt * CH:(nt + 1) * CH])
                return t_

            def xload(nt):
                return (xload_one(xTh, "xh", nt), xload_one(xTl, "xl", nt))

            # chunk-0 loads: interleave wqkh/xh QUARTERS so the (cold)
            # hh pass starts after ~0.5 MB of loads
            hk = KT // 4
            wqkh_sb = wts.tile([P, KT * WQ], f16, tag="wqkh")
            x0h = xin.tile([P, KT * CH], f16, tag="xh", name="xh_0")
            for h in range(4):
                nc.sync.dma_start(
                    wqkh_sb[:, h * hk * WQ:(h + 1) * hk * WQ]
                    .rearrange("p (k c) -> p k c", c=WQ),
                    wqkh.rearrange("(k p) c -> p k c", p=P)
                        [:, h * hk:(h + 1) * hk, :])
                nc.sync.dma_start(
                    x0h[:, h * hk * CH:(h + 1) * hk * CH]
                    .rearrange("p (k c) -> p k c", c=CH),
                    xTh.rearrange("(k p) t -> p k t", p=P)
                       [:, h * hk:(h + 1) * hk, 0:CH])
            x0 = (x0h, xload_one(xTl, "xl", 0))
            wqkl_sb = wts.tile([P, KT * WQ], f16, tag="wqkl")
            for h in range(2):
                hk2 = KT // 2
                nc.sync.dma_start(
                    wqkl_sb[:, h * hk2 * WQ:(h + 1) * hk2 * WQ]
                    .rearrange("p (k c) -> p k c", c=WQ),
                    wqkl.rearrange("(k p) c -> p k c", p=P)
                        [:, h * hk2:(h + 1) * hk2, :])
            wv_sb = wts.tile([P, KT * NHL * HS], f16, tag="wv")
            nc.sync.dma_start(
                wv_sb[:].rearrange("p (k c) -> p k c", c=NHL * HS),
                wv.rearrange("(k p) c -> p k c", p=P))
            bqk_sb = wts.tile([P, 4], f32, tag="bqk")
            nc.sync.dma_start(
                bqk_sb[:].rearrange("p (m o) -> p m o", o=1),
                bqk.rearrange("(m p) o -> p m o", p=P))
            wp_sb = []

            def load_wp():
                for kt in range(NHL * HS // P):  # 2
                    t_ = wts.tile([P, C], f16, tag=f"wp{kt}", name=f"wp{kt}")
                    nc.sync.dma_start(t_[:], wp[kt * P:(kt + 1) * P, :])
                    wp_sb.append(t_)

            # ---- persistent score-operand and value tiles ------------
            q2 = [qkp.tile([P, T], f16, tag=f"q2_{h}", name=f"q2_{h}")
                  for h in range(NHL)]
            kx = [qkp.tile([P, T], f16, tag=f"kx_{h}", name=f"kx_{h}")
                  for h in range(NHL)]
            khh = [qkp.tile([HS, T], f16, tag=f"khh_{h}", name=f"khh_{h}")
                   for h in range(NHL)]
            v_sb = [vsb.tile([P, NHL * HS], f16, tag=f"v{i}", name=f"v{i}")
                    for i in range(T // P)]
            aout_sb = [aout.tile([P, T], f16, tag=f"at{i}", name=f"at{i}")
                       for i in range(2)]

            # ---- phase 1: qkv projections + score-operand assembly ---
            def xs(t_, kt):
                return t_[:, kt * CH:(kt + 1) * CH]

            def extract(nt, mt, ps):
                """Split the [128,512] f32 projection PSUM for column
                tile mt (two heads) into fp16 hi/lo score operands.
                Partition-crossing pieces stage on native partitions
                and move via SBUF->SBUF DMA."""
                cs = slice(nt * CH, (nt + 1) * CH)
                b_e = bqk_sb[0:HS, mt:mt + 1]
                b_o = bqk_sb[HS:P, mt:mt + 1]
                if mt < 2:  # q: heads (2mt, 2mt+1)
                    he, ho = 2 * mt, 2 * mt + 1
                    nc.scalar.activation(q2[he][0:HS, cs], ps[0:HS, :],
                                         AF.Identity, bias=b_e)
                    st = stg.tile([P, CH], f16, tag="stg",
                                  name=f"stg_{nt}_{mt}")
                    nc.vector.scalar_tensor_tensor(
                        st[0:HS, :], ps[0:HS, :], b_e, q2[he][0:HS, cs],
                        op0=ALU.add, op1=ALU.subtract)
                    nc.sync.dma_start(q2[he][HS:P, cs], st[0:HS, :])
                    st2 = stg.tile([P, CH], f16, tag="stg2",
                                   name=f"stg2_{nt}_{mt}")
                    nc.scalar.activation(st2[HS:P, :], ps[HS:P, :],
                                         AF.Identity, bias=b_o)
                    nc.vector.scalar_tensor_tensor(
                        q2[ho][HS:P, cs], ps[HS:P, :], b_o, st2[HS:P, :],
                        op0=ALU.add, op1=ALU.subtract)
                    nc.sync.dma_start(q2[ho][0:HS, cs], st2[HS:P, :])
                else:  # k: heads (2(mt-2), 2(mt-2)+1)
                    he, ho = 2 * (mt - 2), 2 * (mt - 2) + 1
                    nc.scalar.activation(khh[he][:, cs], ps[0:HS, :],
                                         AF.Identity, bias=b_e)
                    nc.vector.scalar_tensor_tensor(
                        kx[he][0:HS, cs], ps[0:HS, :], b_e, khh[he][:, cs],
                        op0=ALU.add, op1=ALU.subtract)
                    nc.sync.dma_start(kx[he][HS:P, cs], khh[he][:, cs])
                    nc.scalar.activation(kx[ho][HS:P, cs], ps[HS:P, :],
                                         AF.Identity, bias=b_o)
                    stk = stg.tile([P, CH], f16, tag="stg",
                                   name=f"stgk_{nt}_{mt}")
                    nc.vector.scalar_tensor_tensor(
                        stk[HS:P, :], ps[HS:P, :], b_o, kx[ho][HS:P, cs],
                        op0=ALU.add, op1=ALU.subtract)
                    nc.sync.dma_start(kx[ho][0:HS, cs], stk[HS:P, :])
                    nc.sync.dma_start(khh[ho][:, cs], kx[ho][HS:P, cs])

            def phase1_chunk(nt):
                xh_, xl_ = x0 if nt == 0 else xload(nt)
                passes = [(wqkh_sb, xh_), (wqkh_sb, xl_), (wqkl_sb, xh_)]
                if nt == 0:
                    # hh pass kt-outer first: starts on the first
                    # wqkh/xh load quarters; hl/lh later per mt
                    pss = [ps_big.tile([P, CH], f32, tag="big",
                                       name=f"p1_0_{mt}") for mt in range(4)]
                    for kt in range(KT):
                        for mt in range(4):
                            nc.tensor.matmul(
                                pss[mt][:],
                                wqkh_sb[:, kt * WQ + mt * P:
                                        kt * WQ + (mt + 1) * P],
                                xs(xh_, kt), start=(kt == 0), stop=False)
                    for mt in range(4):
                        for pi, (wsb, xsb) in enumerate(passes[1:]):
                            for kt in range(KT):
                                nc.tensor.matmul(
                                    pss[mt][:],
                                    wsb[:, kt * WQ + mt * P:
                                        kt * WQ + (mt + 1) * P],
                                    xs(xsb, kt), start=False,
                                    stop=(pi == 1 and kt == KT - 1))
                        extract(nt, mt, pss[mt])
                else:
                    for mt in range(4):
                        ps = ps_big.tile([P, CH], f32, tag="big",
                                         name=f"p1_{nt}_{mt}")
                        for pi, (wsb, xsb) in enumerate(passes):
                            for kt in range(KT):
                                nc.tensor.matmul(
                                    ps[:],
                                    wsb[:, kt * WQ + mt * P:
                                        kt * WQ + (mt + 1) * P],
                                    xs(xsb, kt),
                                    start=(pi == 0 and kt == 0),
                                    stop=(pi == 2 and kt == KT - 1))
                        extract(nt, mt, ps)
                return xh_

            def vproj_tt(xh_, nt, tt):
                ps = ps_o.tile([P, CH], f32, tag="o", name=f"v_{nt}_{tt}")
                for kt in range(KT):
                    nc.tensor.matmul(
                        ps[:, 0:NHL * HS],
                        xh_[:, kt * CH + tt * P:kt * CH + (tt + 1) * P],
                        wv_sb[:, kt * NHL * HS:(kt + 1) * NHL * HS],
                        start=(kt == 0), stop=(kt == KT - 1))
                nc.vector.tensor_copy(v_sb[nt * (CH // P) + tt][:],
                                      ps[:, 0:NHL * HS])

            # ---- phase 2: causal attention -------------------------
            units = {}

            def s_stage(h, qb):
                klen = (qb + 1) * P
                nch = (klen + CH - 1) // CH
                p_sb = att.tile([P, T], f16, tag="P", name=f"p_{h}_{qb}")
                mx = att.tile([P, 4], f32, tag="mx", name=f"mx_{h}_{qb}")
                lp = att.tile([P, 4], f32, tag="lp", name=f"lp_{h}_{qb}")
                qsl = slice(qb * P, (qb + 1) * P)
                order = [nch - 1] + list(range(nch - 1))
                chunk_tiles = {}
                # diag (masked) chunk first: its mask->max chain
                # overlaps the remaining chunks' matmuls
                for kc in order:
                    w = min(CH, klen - kc * CH)
                    ps = ps_big.tile([P, CH], f32, tag="big",
                                     name=f"s_{h}_{qb}_{kc}")
                    ks = slice(kc * CH, kc * CH + w)
                    nc.tensor.matmul(ps[:, :w], q2[h][0:HS, qsl],
                                     khh[h][:, ks], start=True, stop=False)
                    nc.tensor.matmul(ps[:, :w], q2[h][:, qsl],
                                     kx[h][:, ks], start=False, stop=True)
                    if kc == nch - 1:
                        off = w - P
                        nc.vector.tensor_add(ps[:, off:off + P],
                                             ps[:, off:off + P], mask[:])
                    nc.vector.reduce_max(mx[:, kc:kc + 1], ps[:, :w],
                                         axis=AX.X)
                    chunk_tiles[kc] = (ps, w)
                nm = att.tile([P, 1], f32, tag="nm", name=f"nm_{h}_{qb}")
                nc.vector.reduce_max(nm[:], mx[:, :nch], axis=AX.X,
                                     negate=True)
                nmb = att.tile([P, 1], f32, tag="nmb", name=f"nmb_{h}_{qb}")
                nc.vector.tensor_scalar_mul(nmb[:], nm[:], SCALE)
                for kc in order:
                    ps, w = chunk_tiles[kc]
                    nc.scalar.activation(
                        p_sb[:, kc * CH:kc * CH + w], ps[:, :w],
                        AF.Exp, bias=nmb[:], scale=SCALE,
                        accum_out=lp[:, kc:kc + 1])
                l_ = att.tile([P, 1], f32, tag="l", name=f"l_{h}_{qb}")
                nc.vector.reduce_sum(l_[:], lp[:, :nch], axis=AX.X)
                linv = att.tile([P, 1], f32, tag="li", name=f"li_{h}_{qb}")
                nc.vector.reciprocal(linv[:], l_[:])
                # normalize P in place (per-partition 1/l)
                nc.vector.tensor_scalar_mul(p_sb[:, :klen],
                                            p_sb[:, :klen], linv[:])
                # one blocked DMA-xbar transpose: pt[p, j, q] = P[q, j*128+p].
                # All transposes stay on the SP HWDGE queue (concurrent
                # xbar transposes on two rings corrupt each other).
                pt = ptp.tile([P, T], f16, tag="pt", name=f"pt_{h}_{qb}")
                nc.sync.dma_start(
                    pt[:, :klen].rearrange("p (j q) -> p j q", q=P),
                    p_sb[:, :klen], transpose=True)
                units[(h, qb)] = pt

            def pv_stage(pair, qb):
                nkb = qb + 1
                o_ps = ps_o.tile([P, P], f32, tag="o", name=f"o_{pair}_{qb}")
                for i, h in enumerate((2 * pair, 2 * pair + 1)):
                    pt = units.pop((h, qb))
                    rows = slice(0, HS) if i == 0 else slice(HS, P)
                    for kb in range(nkb):
                        nc.tensor.matmul(
                            o_ps[rows, :],
                            v_sb[kb][:, h * HS:(h + 1) * HS],
                            pt[:, kb * P:(kb + 1) * P],
                            start=(kb == 0), stop=(kb == nkb - 1))
                    nc.vector.tensor_copy(
                        aout_sb[pair][rows, qb * P:(qb + 1) * P],
                        o_ps[rows, :])

            # c_proj column tiles stage into one fp16 tile per token
            # range; batched DMA stores (fewer DMAs -> fewer collisions
            # with the serialized transposes)
            proj_state = {}

            def proj_cols(mts, lo, hi, tail=False):
                w = hi - lo
                if lo not in proj_state:
                    proj_state[lo] = [stage.tile([P, 8 * w], f16, tag="stage",
                                                 name=f"stg_{lo}"), 0]
                st, _ = proj_state[lo]
                for mt in mts:
                    ps = ps_big.tile([P, CH], f32, tag="big",
                                     name=f"proj_{mt}_{lo}")
                    for kt in range(2):
                        nc.tensor.matmul(
                            ps[:, 0:w], wp_sb[kt][:, mt * P:(mt + 1) * P],
                            aout_sb[kt][:, lo:hi],
                            start=(kt == 0), stop=(kt == 1))
                    if tail and mt % 2 == 1:
                        nc.vector.tensor_copy(st[:, mt * w:(mt + 1) * w],
                                              ps[:, 0:w])
                    else:
                        nc.scalar.activation(st[:, mt * w:(mt + 1) * w],
                                             ps[:, 0:w], AF.Copy)
                    proj_state[lo][1] += 1
                    if tail and proj_state[lo][1] == 4:
                        nc.sync.dma_start(
                            outT.rearrange("(m p) t -> p m t", p=P)
                                [:, 0:4, lo:hi],
                            st[:, :4 * w].rearrange("p (m c) -> p m c", c=w))
                    if tail and proj_state[lo][1] == 7:
                        nc.sync.dma_start(
                            outT.rearrange("(m p) t -> p m t", p=P)
                                [:, 4:7, lo:hi],
                            st[:, 4 * w:7 * w]
                            .rearrange("p (m c) -> p m c", c=w))
                if proj_state[lo][1] == 8:
                    src = st[:, 7 * w:].rearrange("p (m c) -> p m c", c=w) \
                        if tail else st[:].rearrange("p (m c) -> p m c", c=w)
                    dst = outT.rearrange("(m p) t -> p m t", p=P)
                    nc.sync.dma_start(
                        dst[:, 7:8, lo:hi] if tail else dst[:, :, lo:hi], src)

            proj_sched = {1: (0, 1, 2, 3), 2: (4, 5, 6, 7)}
            for nt in range(NT):
                xh_ = phase1_chunk(nt)
                if nt == 0:
                    load_wp()
                for tt in range(CH // P):
                    vproj_tt(xh_, nt, tt)
                for j in range(4):
                    qb = nt * 4 + j
                    for h in range(NHL):
                        s_stage(h, qb)
                    if qb >= 2:
                        pv_stage(0, qb - 2)
                        pv_stage(1, qb - 2)
                    if nt > 0 and j in proj_sched:
                        proj_cols(proj_sched[j], (nt - 1) * CH, nt * CH)
                    if nt == NT - 1 and j == 3:
                        # tokens [1536,1792) need query blocks 12,13 --
                        # assembled by pv(13) just above
                        proj_cols(range(8), 3 * CH, 3 * CH + 2 * P)
            # flush
            pv_stage(0, 14)
            pv_stage(1, 14)
            pv_stage(0, 15)
            pv_stage(1, 15)
            proj_cols(range(8), 3 * CH + 2 * P, T, tail=True)

    nc.compile()
    return nc


def _get_program():
    if "nc" not in _CACHE:
        _CACHE["nc"] = _build_program()
    return _CACHE["nc"]


def _per_core_inputs(x, w_attn, b_attn, w_proj):
    in_maps = []
    for core in range(NCORES):
        b = core // HG
        hg = core % HG
        xTc = np.ascontiguousarray(x[b].T.astype(np.float32))
        xh = xTc.astype(np.float16)
        xl = (xTc - xh.astype(np.float32)).astype(np.float16)
        qcols = []
        bcols = []
        # q heads then k heads: [q_h0..q_h3 | k_h0..k_h3]
        for off in (0, C):  # q then k
            for j in range(NHL):
                hgl = hg * NHL + j
                qcols.append(w_attn[:, off + hgl * HS: off + (hgl + 1) * HS])
                bcols.append(b_attn[off + hgl * HS: off + (hgl + 1) * HS])
        wqk_ = np.ascontiguousarray(
            np.concatenate(qcols, axis=1).astype(np.float32))
        wqkh_ = wqk_.astype(np.float16)
        wqkl_ = (wqk_ - wqkh_.astype(np.float32)).astype(np.float16)
        bqk_ = np.ascontiguousarray(
            np.concatenate(bcols)[:, None].astype(np.float32))
        wv_ = np.ascontiguousarray(
            w_attn[:, 2 * C + hg * NHL * HS: 2 * C + (hg + 1) * NHL * HS]
            .astype(np.float16))
        wp_ = np.ascontiguousarray(
            w_proj[hg * NHL * HS:(hg + 1) * NHL * HS, :].astype(np.float16))
        in_maps.append({"xTh": xh, "xTl": xl, "wqkh": wqkh_, "wqkl": wqkl_,
                        "bqk": bqk_, "wv": wv_, "wp": wp_})
    return in_maps


def run_sharded(x, w_attn, b_attn, w_proj, b_proj, trace=False, **kw):
    nc = _get_program()
    in_maps = _per_core_inputs(x, w_attn, b_attn, w_proj)
    res = run_bass_kernel_spmd(nc, in_maps, core_ids=list(range(NCORES)),
                               trace=trace, **kw)
    out = np.zeros((B, T, C), dtype=np.float32)
    for core in range(NCORES):
        out[core // HG] += res.results[core]["outT"].T
    corr = (b_attn[2 * C:].astype(np.float32) @ w_proj.astype(np.float32)
            + b_proj.astype(np.float32))
    out += corr[None, None, :]
    return out, res


def kernel(x, w_attn, b_attn, w_proj, b_proj):
    out, _ = run_sharded(np.asarray(x), np.asarray(w_attn), np.asarray(b_attn),
                         np.asarray(w_proj), np.asarray(b_proj))
    return out
